# revision 30
# baseline (speedup 1.0000x reference)
"""Trainium2 Bass kernel for nn_BI3Block (dense_transformer).

Strategy: data-parallel over batch B=8 across the 8 NeuronCores (one batch
element per core, params replicated). Per-core everything is kept in
channel-major layout (channels on SBUF partitions, tokens on the free dim):
 - depthwise convs -> per-partition-scalar multiply/accumulate ops (bf16)
 - BN folded into conv taps/biases on the host
 - channel-mixing GEMMs as lhsT[k=c,*] @ rhs[c,n] in fp32r (bf16 where the
   path is damped: LPE internals, QK^T, PV, FFN2 hidden)
 - softmax without max-subtraction (scores provably in [-0.6, 0.6]); row
   sums come free via a ones-column appended to V
 - LayerNorm stats via ones-vector matmuls + GPSIMD partition_broadcast
A single PE-transpose pass converts the (N,C) inputs to channel-major and the
final result back.
"""

import math
import numpy as np

N = 1024
C = 256
NH = 8
D = 32
MH = 4 * C
HWS = 32
EPS = 1e-5
P = 128
NCORES = 8


def _import_stack():
    try:
        import concourse.bass as bass  # noqa
    except ImportError:
        import sys

        for p in (
            "/opt/trn_rl_repo",
            "/root/.axon_site/_ro/trn_rl_repo",
            "/opt/pypackages",
            "/root/.axon_site/_ro/pypackages",
        ):
            if p not in sys.path:
                sys.path.append(p)
    import concourse.bass as bass
    import concourse.mybir as mybir
    from concourse import tile
    from concourse import bass_utils

    return bass, mybir, tile, bass_utils


def _bacc():
    from concourse import bacc

    return bacc


def _library_config():
    from concourse import library_config

    return library_config


def _bf16(a):
    import ml_dtypes

    return np.ascontiguousarray(np.asarray(a, np.float32).astype(ml_dtypes.bfloat16))


# fixup slots: negated copies of the dx!=0 taps (a-branch idx 0,2,3,5,6,8;
# h-branch idx 9,10,12,13) used to subtract row-wrap contributions
_NEG_SLOTS = {0: 22, 2: 23, 3: 24, 5: 25, 6: 26, 8: 27, 9: 28, 10: 29, 12: 30, 13: 31}


# ---------------------------------------------------------------- host prep
def _prep_params(params):
    f32 = np.float32

    def A(x):
        return np.ascontiguousarray(np.asarray(x), dtype=f32)

    # LPE: fold BN scale into the depthwise taps, BN shift+conv bias into the
    # gelu bias. taps layout per channel: [a(9), h(5), v(5), bias_a, bias_h,
    # bias_v]  -> (2, C, 22)
    taps = np.zeros((2, C, 32), f32)
    fwt = np.zeros((2 * 3 * C, C), f32)
    fbias = np.zeros((2, C), f32)
    for l, key in enumerate(("lpe_q", "lpe_k")):
        p = {k: A(v) for k, v in params[key].items()}
        sc = p["bn_g"] / np.sqrt(p["bn_v"] + EPS)
        sh = p["bn_b"] - p["bn_m"] * sc
        sa, sb_, sv = sc[:C], sc[C : 2 * C], sc[2 * C :]
        ha, hb, hv = sh[:C], sh[C : 2 * C], sh[2 * C :]
        w3 = p["w3"][:, 0]  # (C,3,3) index [c, dy+1, dx+1]
        taps[l, :, 0:9] = w3.reshape(C, 9) * sa[:, None]
        taps[l, :, 9:14] = p["w15"][:, 0, 0] * sb_[:, None]  # (C,5) dx+2
        taps[l, :, 14:19] = p["w51"][:, 0, :, 0] * sv[:, None]  # (C,5) dy+2
        taps[l, :, 19] = p["b3"] * sa + ha
        taps[l, :, 20] = p["b15"] * sb_ + hb
        taps[l, :, 21] = p["b51"] * sv + hv
        for idx, slot in _NEG_SLOTS.items():
            taps[l, :, slot] = -taps[l, :, idx]
        fwt[l * 3 * C : (l + 1) * 3 * C] = p["fw"].T  # (3C, C)
        fbias[l] = p["fb"]

    g = {k: A(v) for k, v in params["gdfa"].items()}
    dm_sc = g["dm_g"] / np.sqrt(g["dm_v"] + EPS)
    dm_sh = g["dm_beta"] - g["dm_m"] * dm_sc
    wqkvm = np.stack(
        [
            (g["dm_w"] * dm_sc[:, None]).T,  # lhsT (c_in, c_out)
            g["wq"],
            g["wk"],
            g["wv"],
        ]
    )
    bqkvm = np.stack([g["dm_b"] * dm_sc + dm_sh, g["bq"], g["bk"], g["bv"]])
    bvb = np.broadcast_to(g["bv"], (P, C)).copy()

    f = params["ffn"]
    ln = np.stack(
        [
            np.stack([A(params["ln1_g"]), A(params["ln1_b"])]),
            np.stack([A(params["ln2_g"]), A(params["ln2_b"])]),
        ]
    )
    out = dict(
        lpetaps=A(taps),
        fwt=_bf16(fwt),
        fbias=A(fbias),
        wqkvm=A(wqkvm),
        bqkvm=A(bqkvm),
        bvb=A(bvb),
        wp=A(g["wp"]),
        bp=A(g["bp"]),
        w1=A(f["w1"]),
        b1=A(f["b1"]),
        w2=_bf16(f["w2"]),
        b2=A(f["b2"]),
        ln=A(ln),
        ident=np.eye(P, dtype=f32),
        onesm=np.ones((P, P), dtype=f32),
    )
    return out


# ---------------------------------------------------------------- bass build
def build_bass():
    bass, mybir, tile, _ = _import_stack()
    f32 = mybir.dt.float32
    f32r = mybir.dt.float32r
    bf16 = mybir.dt.bfloat16
    OP = mybir.AluOpType
    AF = mybir.ActivationFunctionType

    nc = _bacc().Bacc("TRN2", target_bir_lowering=False)
    EI, EO = "ExternalInput", "ExternalOutput"
    dx1 = nc.dram_tensor("x1", (N, C), f32, kind=EI)
    dx2 = nc.dram_tensor("x2", (N, C), f32, kind=EI)
    dtaps = nc.dram_tensor("lpetaps", (2, C, 32), f32, kind=EI)
    dfwt = nc.dram_tensor("fwt", (2 * 3 * C, C), bf16, kind=EI)
    dfb = nc.dram_tensor("fbias", (2, C), f32, kind=EI)
    dwqkvm = nc.dram_tensor("wqkvm", (4, C, C), f32r, kind=EI)
    dbqkvm = nc.dram_tensor("bqkvm", (4, C), f32, kind=EI)
    dbvb = nc.dram_tensor("bvb", (P, C), f32, kind=EI)
    dwp = nc.dram_tensor("wp", (C, C), f32r, kind=EI)
    dbp = nc.dram_tensor("bp", (C,), f32, kind=EI)
    dw1 = nc.dram_tensor("w1", (C, MH), f32r, kind=EI)
    db1 = nc.dram_tensor("b1", (MH,), f32, kind=EI)
    dw2 = nc.dram_tensor("w2", (MH, C), bf16, kind=EI)
    db2 = nc.dram_tensor("b2", (C,), f32, kind=EI)
    dln = nc.dram_tensor("ln", (2, 2, C), f32, kind=EI)
    dident = nc.dram_tensor("ident", (P, P), f32, kind=EI)
    donesm = nc.dram_tensor("onesm", (P, P), f32r, kind=EI)
    dout = nc.dram_tensor("out", (N, C), f32, kind=EO)

    def mm(out, lhsT, rhs, start=True, stop=True, tile_position=None):
        if lhsT.dtype == f32:
            lhsT = lhsT.bitcast(f32r)
        if rhs.dtype == f32:
            rhs = rhs.bitcast(f32r)
        nc.tensor.matmul(
            out, lhsT, rhs, start=start, stop=stop, tile_position=tile_position
        )

    with tile.TileContext(nc) as tc:
        with (
            nc.allow_low_precision(reason="fp32r rounding is intentional"),
            tc.tile_pool(name="wts", bufs=1) as wpool,
            tc.tile_pool(name="per", bufs=1) as per,
            tc.tile_pool(name="scr", bufs=6) as scr,  # f32 [P,N] scratch
            tc.tile_pool(name="catb", bufs=9) as pcat,  # bf16 [P,N] scratch
            tc.tile_pool(name="small", bufs=2) as psm,
            tc.tile_pool(name="epool", bufs=12) as ep,
            tc.tile_pool(name="psmm", bufs=2, space="PSUM") as psmm,
            tc.tile_pool(name="pspv", bufs=3, space="PSUM") as pspv,
            tc.tile_pool(name="pstr", bufs=2, space="PSUM") as pstr,
            nc.psum_tensor([P, P], f32) as warm_ps,
        ):
            # ---- weight DMAs
            def stage(dram_ap, shape, tag, dt=f32):
                t = wpool.tile(shape, dt, tag=tag, name=tag)
                nc.sync.dma_start(t[...], dram_ap)
                return t

            taps_sb = stage(
                dtaps[:, :, :].rearrange("l (q p) s -> p l q s", p=P),
                [P, 2, 2, 32],
                "taps",
            )
            fwt_sb = stage(
                dfwt[:, :].rearrange("(t p) m -> p t m", p=P),
                [P, 12, C],
                "fwt",
                bf16,
            )
            fb_sb = stage(
                dfb[:, :].rearrange("l (q p) -> p l q", p=P), [P, 2, 2], "fb"
            )
            wqkv_sb = stage(
                dwqkvm[:, :, :].rearrange("m (k p) n -> p m k n", p=P),
                [P, 4, 2, C],
                "wqkv",
                f32r,
            )
            bqkv_sb = stage(
                dbqkvm[:, :].rearrange("m (q p) -> p m q", p=P), [P, 4, 2], "bqkv"
            )
            bvb_sb = stage(dbvb[:, :], [P, C], "bvb")
            wp_sb = stage(
                dwp[:, :].rearrange("(k p) n -> p k n", p=P),
                [P, 2, C],
                "wp",
                f32r,
            )
            bp_sb = stage(dbp[:].rearrange("(q p) -> p q", p=P), [P, 2], "bp")
            w1_sb = stage(
                dw1[:, :].rearrange("(k p) n -> p k n", p=P),
                [P, 2, MH],
                "w1",
                f32r,
            )
            b1_sb = stage(db1[:].rearrange("(q p) -> p q", p=P), [P, 8], "b1")
            w2_sb = stage(
                dw2[:, :].rearrange("(k p) n -> p k n", p=P),
                [P, 8, C],
                "w2",
                bf16,
            )
            b2_sb = stage(db2[:].rearrange("(q p) -> p q", p=P), [P, 2], "b2")
            ln_sb = stage(
                dln[:, :, :].rearrange("a b (q p) -> p a b q", p=P),
                [P, 2, 2, 2],
                "ln",
            )
            ident_sb = stage(dident[:, :], [P, P], "ident")
            onesb_t = stage(donesm[:, :], [P, P], "onesb", f32r)
            onesb_sb = onesb_t
            ones_sb = onesb_t[:, 0:1]
            eps_sb = wpool.tile([1, 1], f32, tag="eps", name="eps")
            nc.vector.memset(eps_sb[...], EPS)

            def T(pool, shape, tag, dt=f32, bufs=None):
                return pool.tile(shape, dt, tag=tag, name=tag, bufs=bufs)

            # Pre-touch ident on PE: transpose matmuls carry a single wait
            # slot (LDW-only), so the real transposes must not need to wait
            # on the ident DMA — PE program order covers it after this.
            def absorb(in_ap):
                nc.tensor.transpose(warm_ps[:, :], in_ap, ident_sb[...])

            absorb(ident_sb[...])

            # ---- load x1/x2, transpose to channel-major, make bf16 copies
            xT = {}
            xTb = {}
            for l, dx in ((0, dx1), (1, dx2)):
                xtm = T(psm, [P, 8, C], "xtm", bufs=2)
                nc.sync.dma_start(
                    xtm[...], dx[:, :].rearrange("(t p) c -> p t c", p=P)
                )
                absorb(xtm[:, 0, 0:P])
                for ct in range(2):
                    t = T(scr, [P, N], "xT", bufs=6)
                    xT[l, ct] = t
                    for nt in range(8):
                        pst = T(pstr, [P, P], "tr")
                        nc.tensor.transpose(
                            pst[...],
                            xtm[:, nt, ct * P : (ct + 1) * P],
                            ident_sb[...],
                        )
                        nc.scalar.activation(
                            t[:, nt * P : (nt + 1) * P], pst[...], AF.Copy
                        )
                    tb = T(pcat, [P, N], "catb", bf16)
                    xTb[l, ct] = tb
                    nc.vector.tensor_copy(tb[...], t[...])

            # ---- LPE per input -> xq[l][ct] = x + lpe(x)   (channel-major)
            # branch a (3x3) + v (5x1) on DVE, branch h (1x5) on GPSIMD
            xq = {}
            for l in range(2):
                cat = {}
                for ct in range(2):
                    xf = xTb[l, ct]
                    xv = xf[:, :].rearrange("p (h w) -> p h w", w=HWS)
                    ca = T(pcat, [P, N], "catb", bf16)
                    ch_ = T(pcat, [P, N], "catb", bf16)
                    cv = T(pcat, [P, N], "catb", bf16)
                    cat[0, ct], cat[1, ct], cat[2, ct] = ca, ch_, cv

                    def tap_ap(idx):
                        return taps_sb[:, l, ct, idx : idx + 1]

                    # center taps first (full-range writes)
                    nc.vector.tensor_scalar_mul(ca[...], xf[...], tap_ap(4))
                    nc.vector.tensor_scalar_mul(ch_[...], xf[...], tap_ap(11))
                    nc.vector.tensor_scalar_mul(cv[...], xf[...], tap_ap(16))

                    def acc(dst, dy, dx, idx):
                        h0, h1 = max(0, -dy), HWS - max(0, dy)
                        w0, w1 = max(0, -dx), HWS - max(0, dx)
                        dv = dst[:, :].rearrange("p (h w) -> p h w", w=HWS)
                        o = dv[:, h0:h1, w0:w1]
                        i = xv[:, h0 + dy : h1 + dy, w0 + dx : w1 + dx]
                        nc.vector.scalar_tensor_tensor(
                            o, i, tap_ap(idx), o, OP.mult, OP.add
                        )

                    for dy in (-1, 0, 1):
                        for dx in (-1, 0, 1):
                            if dy == 0 and dx == 0:
                                continue
                            acc(ca, dy, dx, (dy + 1) * 3 + (dx + 1))
                    for dx in (-2, -1, 1, 2):
                        acc(ch_, 0, dx, 9 + dx + 2)
                    for dy in (-2, -1, 1, 2):
                        acc(cv, dy, 0, 14 + dy + 2)

                # gelu(bn(cat)) in place; bias carries conv-bias + bn shift
                for br in range(3):
                    for ct in range(2):
                        t = cat[br, ct]
                        nc.scalar.activation(
                            t[...],
                            t[...],
                            AF.Gelu,
                            bias=taps_sb[:, l, ct, 19 + br : 20 + br],
                            scale=1.0,
                        )

                # fw matmul (K=3C, bf16) + fb bias + residual -> xq (f32)
                for ct in range(2):
                    t = T(per, [P, N], f"xq{l}{ct}", f32r, bufs=1)
                    xq[l, ct] = t
                    for ch in range(2):
                        ps = T(psmm, [P, 512], "mm")
                        for kt in range(6):
                            br, ck = divmod(kt, 2)
                            mm(
                                ps[...],
                                fwt_sb[:, l * 6 + kt, ct * P : (ct + 1) * P],
                                cat[br, ck][:, ch * 512 : (ch + 1) * 512],
                                start=(kt == 0),
                                stop=(kt == 5),
                            )
                        nc.vector.scalar_tensor_tensor(
                            t[:, ch * 512 : (ch + 1) * 512],
                            ps[...],
                            fb_sb[:, l, ct : ct + 1],
                            xT[l, ct][:, ch * 512 : (ch + 1) * 512],
                            OP.add,
                            OP.add,
                        )

            # ---- GDFA: mod = q*(k-q); kv = relu(dmw @ mod + dmb)
            kvt = {}
            modt = {}
            for ct in range(2):
                t = T(scr, [P, N], "xT", f32r, bufs=6)
                modt[ct] = t
                nc.vector.tensor_sub(t[...], xq[1, ct][...], xq[0, ct][...])
                nc.vector.tensor_mul(t[...], t[...], xq[0, ct][...])
            for mt in range(2):
                t = T(scr, [P, N], "xT", f32r, bufs=6)
                kvt[mt] = t
                for ch in range(2):
                    ps = T(psmm, [P, 512], "mm")
                    for kt in range(2):
                        mm(
                            ps[...],
                            wqkv_sb[:, 0, kt, mt * P : (mt + 1) * P],
                            modt[kt][:, ch * 512 : (ch + 1) * 512],
                            start=(kt == 0),
                            stop=(kt == 1),
                        )
                    nc.vector.tensor_scalar(
                        t[:, ch * 512 : (ch + 1) * 512],
                        ps[...],
                        bqkv_sb[:, 0, mt : mt + 1],
                        0.0,
                        OP.add,
                        OP.max,
                    )

            # ---- projections: qp/kp channel-major bf16; v token-major + ones
            qp, kp = {}, {}
            for name, dst, srcd, mi in (("q", qp, None, 1), ("k", kp, None, 2)):
                for mt in range(2):
                    t = T(pcat, [P, N], "catb", bf16)
                    dst[mt] = t
                    for ch in range(2):
                        ps = T(psmm, [P, 512], "mm")
                        for kt in range(2):
                            s = xq[0, kt] if name == "q" else kvt[kt]
                            mm(
                                ps[...],
                                wqkv_sb[:, mi, kt, mt * P : (mt + 1) * P],
                                s[:, ch * 512 : (ch + 1) * 512],
                                start=(kt == 0),
                                stop=(kt == 1),
                            )
                        nc.vector.tensor_scalar_add(
                            t[:, ch * 512 : (ch + 1) * 512],
                            ps[...],
                            bqkv_sb[:, mi, mt : mt + 1],
                        )

            vext = {}
            for nb in range(8):
                t = T(per, [P, NH, D + 1], f"vext{nb}", bf16, bufs=1)
                vext[nb] = t
                ps = T(psmm, [P, C], "mm")
                for kt in range(2):
                    mm(
                        ps[...],
                        kvt[kt][:, nb * P : (nb + 1) * P],
                        wqkv_sb[:, 3, kt, :],
                        start=(kt == 0),
                        stop=(kt == 1),
                    )
                nc.vector.tensor_tensor(
                    t[:, :, 0:D],
                    ps[:, :].rearrange("p (h d) -> p h d", d=D),
                    bvb_sb[:, :].rearrange("p (h d) -> p h d", d=D),
                    OP.add,
                )
                nc.vector.memset(t[:, :, D : D + 1], 1.0)

            # ---- attention per head
            SM = 1.0 / math.sqrt(D)
            ofull = {
                ct: T(per, [P, N], f"of{ct}", f32r, bufs=1) for ct in range(2)
            }
            def emit_qk(h):
                cth, ro = h // 4, (h % 4) * D
                et = {}
                for mb in range(8):
                    e = T(ep, [P, N], "E", bf16)
                    et[mb] = e
                    for ch in range(2):
                        ps = T(psmm, [P, 512], "mm")
                        mm(
                            ps[...],
                            kp[cth][ro : ro + D, mb * P : (mb + 1) * P],
                            qp[cth][ro : ro + D, ch * 512 : (ch + 1) * 512],
                            tile_position=(ro, 0),
                        )
                        nc.scalar.activation(
                            e[:, ch * 512 : (ch + 1) * 512],
                            ps[...],
                            AF.Exp,
                            bias=0.0,
                            scale=SM,
                        )
                return et

            ets = {0: emit_qk(0)}
            for h in range(NH):
                cth, ro = h // 4, (h % 4) * D
                if h + 1 < NH:
                    ets[h + 1] = emit_qk(h + 1)
                et = ets.pop(h)
                for ch in range(2):
                    po = T(pspv, [P, 512], "pv")
                    for kt in range(8):
                        mm(
                            po[0 : D + 1, :],
                            vext[kt][:, h, :],
                            et[kt][:, ch * 512 : (ch + 1) * 512],
                            start=(kt == 0),
                            stop=(kt == 7),
                        )
                    r = T(psm, [D + 1, 512], "r", f32r)
                    nc.vector.reciprocal(r[D : D + 1, :], po[D : D + 1, :])
                    bc = T(psmm, [P, 512], "mm")
                    mm(
                        bc[0:D, :],
                        onesb_sb[D : D + 1, 0:D],
                        r[D : D + 1, :],
                        tile_position=(D, 0),
                    )
                    bcs = T(psm, [D, 512], "bcs")
                    nc.scalar.activation(bcs[...], bc[0:D, :], AF.Copy)
                    on = T(psm, [D, 512], "on", f32r)
                    nc.vector.tensor_mul(on[...], po[0:D, :], bcs[...])
                    nc.sync.dma_start(
                        ofull[cth][ro : ro + D, ch * 512 : (ch + 1) * 512],
                        on[...],
                    )

            # ---- layernorm helper (channel-major, stats via ones matmul)
            def layer_norm(src, li, out_tag, out_dt=f32):
                sq = {}
                for mt in range(2):
                    s = T(scr, [P, N], "xT", f32r, bufs=6)
                    sq[mt] = s
                    nc.scalar.activation(s[...], src[mt][...], AF.Square)
                mean = T(psm, [1, N], "mean", f32r, bufs=1)
                var = T(psm, [1, N], "var", f32r, bufs=1)
                for ch in range(2):
                    sl = slice(ch * 512, (ch + 1) * 512)
                    psA = T(psmm, [1, 512], "mm")
                    for kt in range(2):
                        mm(
                            psA[...],
                            ones_sb[...],
                            src[kt][:, sl],
                            start=(kt == 0),
                            stop=(kt == 1),
                        )
                    nc.vector.tensor_scalar_mul(mean[:, sl], psA[...], 1.0 / C)
                    psB = T(psmm, [1, 512], "mm")
                    for kt in range(2):
                        mm(
                            psB[...],
                            ones_sb[...],
                            sq[kt][:, sl],
                            start=(kt == 0),
                            stop=(kt == 1),
                        )
                    nc.vector.tensor_scalar_mul(var[:, sl], psB[...], 1.0 / C)
                # var = E[x^2] - mean^2 ; rstd = 1/sqrt(var+eps)
                m2 = T(psm, [1, N], "m2", f32r, bufs=1)
                nc.vector.tensor_mul(m2[...], mean[...], mean[...])
                nc.vector.tensor_sub(var[...], var[...], m2[...])
                nc.scalar.activation(
                    var[...], var[...], AF.Sqrt, bias=eps_sb[0:1, 0:1]
                )
                nc.vector.reciprocal(var[...], var[...])
                outs = {
                    mt: T(per, [P, N], f"{out_tag}{mt}", out_dt, bufs=1)
                    for mt in range(2)
                }
                for ch in range(2):
                    sl = slice(ch * 512, (ch + 1) * 512)
                    mb = T(psmm, [P, 512], "mm")
                    mm(mb[...], onesb_sb[0:1, 0:P], mean[0:1, sl])
                    rb = T(psmm, [P, 512], "mm")
                    mm(rb[...], onesb_sb[0:1, 0:P], var[0:1, sl])
                    for mt in range(2):
                        o = outs[mt]
                        nc.vector.tensor_sub(o[:, sl], src[mt][:, sl], mb[...])
                        nc.vector.tensor_mul(o[:, sl], o[:, sl], rb[...])
                        nc.vector.tensor_scalar(
                            o[:, sl],
                            o[:, sl],
                            ln_sb[:, li, 0, mt : mt + 1],
                            ln_sb[:, li, 1, mt : mt + 1],
                            OP.mult,
                            OP.add,
                        )
                return outs

            # ---- wp + residual + LN1
            attres = {}
            for mt in range(2):
                t = T(scr, [P, N], "xT", f32r, bufs=6)
                attres[mt] = t
                for ch in range(2):
                    ps = T(psmm, [P, 512], "mm")
                    for kt in range(2):
                        mm(
                            ps[...],
                            wp_sb[:, kt, mt * P : (mt + 1) * P],
                            ofull[kt][:, ch * 512 : (ch + 1) * 512],
                            start=(kt == 0),
                            stop=(kt == 1),
                        )
                    nc.vector.scalar_tensor_tensor(
                        t[:, ch * 512 : (ch + 1) * 512],
                        ps[...],
                        bp_sb[:, mt : mt + 1],
                        xq[0, mt][:, ch * 512 : (ch + 1) * 512],
                        OP.add,
                        OP.add,
                    )
            xattn = layer_norm(attres, 0, "xattn", f32r)

            # ---- FFN (hidden in bf16, chunked over n)
            ffnres = {
                mt: T(scr, [P, N], "xT", f32r, bufs=6) for mt in range(2)
            }
            for ch in range(2):
                hcm = {}
                for mh in range(8):
                    t = T(psm, [P, 512], "hcm", bf16, bufs=9)
                    hcm[mh] = t
                    ps = T(psmm, [P, 512], "mm")
                    for kt in range(2):
                        mm(
                            ps[...],
                            w1_sb[:, kt, mh * P : (mh + 1) * P],
                            xattn[kt][:, ch * 512 : (ch + 1) * 512],
                            start=(kt == 0),
                            stop=(kt == 1),
                        )
                    nc.scalar.activation(
                        t[...],
                        ps[...],
                        AF.Gelu,
                        bias=b1_sb[:, mh : mh + 1],
                        scale=1.0,
                    )
                for mt in range(2):
                    ps = T(psmm, [P, 512], "mm")
                    for kt in range(8):
                        mm(
                            ps[...],
                            w2_sb[:, kt, mt * P : (mt + 1) * P],
                            hcm[kt][...],
                            start=(kt == 0),
                            stop=(kt == 7),
                        )
                    nc.vector.scalar_tensor_tensor(
                        ffnres[mt][:, ch * 512 : (ch + 1) * 512],
                        ps[...],
                        b2_sb[:, mt : mt + 1],
                        xattn[mt][:, ch * 512 : (ch + 1) * 512],
                        OP.add,
                        OP.add,
                    )
            fin = layer_norm(ffnres, 1, "fin")

            # ---- transpose back to token-major and store
            dview = dout[:, :].rearrange("(t p) c -> p t c", p=P)
            absorb(fin[0][:, 0:P])
            absorb(fin[1][:, 0:P])
            absorb(fin[0][:, 512 : 512 + P])
            absorb(fin[1][:, 512 : 512 + P])
            for nt in range(8):
                ot = T(psm, [P, C], "otm", bufs=2)
                for mt in range(2):
                    pst = T(pstr, [P, P], "tr")
                    nc.tensor.transpose(
                        pst[...],
                        fin[mt][:, nt * P : (nt + 1) * P],
                        ident_sb[...],
                    )
                    nc.vector.tensor_copy(
                        ot[:, mt * P : (mt + 1) * P], pst[...]
                    )
                nc.sync.dma_start(dview[:, nt, :], ot[...])

    nc.compile()
    return nc


_CACHE = {}


def kernel(x1, x2, x3=None, params=None, **_):
    bass, mybir, tile, bass_utils = _import_stack()
    x1 = np.ascontiguousarray(np.asarray(x1), dtype=np.float32)
    x2 = np.ascontiguousarray(np.asarray(x2), dtype=np.float32)
    pp = _prep_params(params)
    if "nc" not in _CACHE:
        _CACHE["nc"] = build_bass()
    nc = _CACHE["nc"]
    in_maps = []
    for b in range(NCORES):
        m = dict(pp)
        m["x1"] = np.ascontiguousarray(x1[b])
        m["x2"] = np.ascontiguousarray(x2[b])
        in_maps.append(m)
    res = bass_utils.run_bass_kernel_spmd(
        nc, in_maps, core_ids=list(range(NCORES))
    )
    out = np.stack([np.asarray(r["out"]) for r in res.results])
    return out.astype(np.float32)
